# revision 4
# baseline (speedup 1.0000x reference)
"""Causal sliding-window attention (window=256, temperature=8) on Trainium2.

Problem: q,k,v [B=2, H=16, S=2048, D=64] f32.  Returns (out, attn) like the
reference: out = softmax(mask(QK^T/8)) @ V and attn = the full [S, S]
probability matrix (exactly zero outside the causal 256-wide band).

Sharding: the 32 (b,h) pairs are data-parallel; each of the 8 NeuronCores
processes 4 heads end-to-end (no cross-core communication).

Per head on-device:
  - load q,k,v; PE-transpose q,k into [D, S] layout (contraction dim on
    partitions for the QK^T matmuls).
  - per 128-row query tile t: the allowed keys live in block columns
    [t-2, t-1, t] (at most 384 keys).  One matmul produces the [128, <=384]
    score tile; additive -1e9 band mask; Exp on the scalar engine with
    fused row-sum (softmax denominator); reciprocal; normalize.
  - the normalized probability tile is DMA'd out compactly
    (band[head, t, :, :wN]) and PE-transposed so P@V can run as
    matmul(lhsT=P^T, rhs=V) accumulating over the <=3 key blocks.

Host side: scatters band tiles into the zero [S, S] matrix (rows are
disjoint across tiles; masked entries inside a tile are exactly 0, matching
the reference's exp(-1e9 - max) underflow).
"""

import json

import numpy as np

import concourse.bass as bass
import concourse.bass2jax as bass2jax
import concourse.bass_utils as bass_utils
import concourse.mybir as mybir
import concourse.tile as tile
from concourse.bass import ts
from concourse.bass_utils import run_bass_kernel_spmd
from concourse.masks import make_identity

# ---------------------------------------------------------------------------
# Wait legalization: the walrus build in this container accepts only ONE
# sync wait per instruction (setupSyncWait: "Too many sync wait commands"),
# but Tile's semaphore assignment freely attaches several.  Split every
# excess wait into a standalone single-wait EventSemaphore instruction on
# the same engine immediately before the real instruction (the engine's
# sequencer blocks on each in turn — semantics preserved).
# ---------------------------------------------------------------------------

_MAX_WAITS = 1


def _legalize_block(bb, counter):
    out_instrs = []
    for ins in bb.get("instructions", []):
        si = ins.get("sync_info")
        waits = (si or {}).get("on_wait") or []
        if len(waits) > _MAX_WAITS:
            keep = waits[-_MAX_WAITS:]
            hoist = waits[:-_MAX_WAITS]
            for w in hoist:
                counter[0] += 1
                out_instrs.append({
                    "debug": ins.get("debug", 0),
                    "engine": ins["engine"],
                    "ins": [],
                    "name": f"evw_{counter[0]}_{ins['name']}",
                    "opcode": "EventSemaphore",
                    "outs": [],
                    "sync_info": {"on_update": [], "on_wait": [w]},
                })
            si["on_wait"] = keep
        out_instrs.append(ins)
    bb["instructions"] = out_instrs
    for sub in bb.get("blocks", []):
        _legalize_block(sub, counter)


def _legalize_waits(bir_json):
    if isinstance(bir_json, bytes):
        d = json.loads(bir_json)
    else:
        d = json.loads(bir_json)
    counter = [0]
    for f in d.get("functions", []):
        for bb in f.get("blocks", []):
            _legalize_block(bb, counter)
    return json.dumps(d).encode()


_orig_compile_bir_kernel = bass_utils.compile_bir_kernel


def _patched_compile_bir_kernel(bir_json, tmpdir, neff_name="file.neff"):
    return _orig_compile_bir_kernel(_legalize_waits(bir_json), tmpdir,
                                    neff_name=neff_name)


if getattr(bass_utils.compile_bir_kernel, "__name__", "") != "_patched_compile_bir_kernel":
    bass_utils.compile_bir_kernel = _patched_compile_bir_kernel
    bass2jax.compile_bir_kernel = _patched_compile_bir_kernel

F32 = mybir.dt.float32
P = 128          # partition / query tile rows
D = 64           # head dim
S = 2048         # sequence length
T = S // P       # 16 query tiles per head
W = 3 * P        # widest key window per query tile (2 prev blocks + diag)
NH = 4           # (b, h) pairs per core
NCORES = 8
TEMP = 8.0
NEG = -1e9


def _build(repeat=1):
    nc = bass.Bass("TRN2", target_bir_lowering=False, debug=False)
    q = nc.dram_tensor("q", [NH, S, D], F32, kind="ExternalInput").ap()
    k = nc.dram_tensor("k", [NH, S, D], F32, kind="ExternalInput").ap()
    v = nc.dram_tensor("v", [NH, S, D], F32, kind="ExternalInput").ap()
    out = nc.dram_tensor("out", [NH, S, D], F32, kind="ExternalOutput").ap()
    band = nc.dram_tensor("band", [NH, T, P, W], F32, kind="ExternalOutput").ap()

    with tile.TileContext(nc) as tc:
        with (
            tc.tile_pool(name="consts", bufs=1) as consts,
            tc.tile_pool(name="perhead", bufs=2) as perhead,
            tc.tile_pool(name="work", bufs=3) as work,
            tc.tile_pool(name="pwork", bufs=2) as pwork,
            tc.tile_pool(name="ps_tr", bufs=2, space="PSUM") as ps_tr,
            tc.tile_pool(name="ps_sc", bufs=2, space="PSUM") as ps_sc,
            tc.tile_pool(name="ps_tp", bufs=2, space="PSUM") as ps_tp,
            tc.tile_pool(name="ps_o", bufs=2, space="PSUM") as ps_o,
        ):
            ident = consts.tile([P, P], F32)
            make_identity(nc, ident)

            # Additive mask for a [128, 384] score tile whose columns are key
            # blocks [t-2, t-1, t].  Block t-2: row r allows cols jj > r.
            # Block t-1: fully allowed.  Diag block t: allows jj <= r.
            mask = consts.tile([P, W], F32)
            nc.gpsimd.memset(mask, 0.0)
            nc.gpsimd.affine_select(
                mask[:, 0:P], mask[:, 0:P],
                compare_op=mybir.AluOpType.is_gt,
                fill=NEG, base=0, channel_multiplier=-1, pattern=[[1, P]],
            )
            nc.gpsimd.affine_select(
                mask[:, 2 * P:3 * P], mask[:, 2 * P:3 * P],
                compare_op=mybir.AluOpType.is_ge,
                fill=NEG, base=0, channel_multiplier=1, pattern=[[-1, P]],
            )

            for hd in [h for _ in range(repeat) for h in range(NH)]:
                q_nat = perhead.tile([P, T, D], F32, tag="qnat")
                k_nat = perhead.tile([P, T, D], F32, tag="knat")
                v_sb = perhead.tile([P, T, D], F32, tag="v")
                nc.sync.dma_start(q_nat, q[hd].rearrange("(t p) d -> p t d", p=P))
                nc.sync.dma_start(k_nat, k[hd].rearrange("(t p) d -> p t d", p=P))
                nc.sync.dma_start(v_sb, v[hd].rearrange("(t p) d -> p t d", p=P))

                # Transpose q, k tiles into [D, t, 128] so the QK^T contraction
                # (over D) sits on the partition dimension.
                qT = perhead.tile([D, T, P], F32, tag="qT")
                kT = perhead.tile([D, T, P], F32, tag="kT")
                for t4 in range(T // 4):
                    tq = ps_tr.tile([D, 4, P], F32, tag="tqk")
                    tk = ps_tr.tile([D, 4, P], F32, tag="tqk")
                    for j in range(4):
                        t = 4 * t4 + j
                        nc.tensor.transpose(tq[:, j], q_nat[:, t, :], ident)
                        nc.tensor.transpose(tk[:, j], k_nat[:, t, :], ident)
                    nc.scalar.copy(qT[:, ts(t4, 4), :], tq)
                    nc.vector.tensor_copy(kT[:, ts(t4, 4), :], tk)

                o_all = perhead.tile([P, T, D], F32, tag="o")

                for t in range(T):
                    w0b = max(0, t - 2)     # first key block of the window
                    nb = t + 1 - w0b        # 1, 2 or 3 key blocks
                    wN = nb * P

                    sc = ps_sc.tile([P, W], F32, tag="sc")
                    nc.tensor.matmul(
                        sc[:, :wN], qT[:, t, :], kT[:, w0b:w0b + nb, :],
                        start=True, stop=True,
                    )
                    nc.vector.tensor_add(sc[:, :wN], sc[:, :wN], mask[:, W - wN:])

                    p_sb = work.tile([P, W], F32, tag="p")
                    den = work.tile([P, 1], F32, tag="den")
                    recip = work.tile([P, 1], F32, tag="recip")
                    nc.scalar.activation(
                        p_sb[:, :wN], sc[:, :wN],
                        mybir.ActivationFunctionType.Exp,
                        scale=1.0 / TEMP, accum_out=den,
                    )
                    nc.vector.reciprocal(recip, den)
                    nc.scalar.mul(p_sb[:, :wN], p_sb[:, :wN], recip)

                    nc.sync.dma_start(band[hd, t, :, :wN], p_sb[:, :wN])

                    # P @ V: transpose P so the key contraction is on partitions.
                    tp = ps_tp.tile([P, 3, P], F32, tag="tp")
                    for c in range(nb):
                        nc.tensor.transpose(tp[:, c], p_sb[:, ts(c, P)], ident)
                    pT = pwork.tile([P, 3, P], F32, tag="pT")
                    nc.vector.tensor_copy(pT[:, :nb, :], tp[:, :nb, :])

                    o_ps = ps_o.tile([P, D], F32, tag="o")
                    for c in range(nb):
                        nc.tensor.matmul(
                            o_ps, pT[:, c, :], v_sb[:, w0b + c, :],
                            start=(c == 0), stop=(c == nb - 1),
                        )
                    nc.vector.tensor_copy(o_all[:, t, :], o_ps)

                nc.sync.dma_start(out[hd].rearrange("(t p) d -> p t d", p=P), o_all)
    return nc


LAST_RESULTS = None


def _run(in_maps, trace=False):
    global LAST_RESULTS
    nc = _build()
    LAST_RESULTS = run_bass_kernel_spmd(
        nc, in_maps, core_ids=list(range(NCORES)), trace=trace,
    )
    return LAST_RESULTS.results


def kernel(q, k, v, _trace=False):
    q = np.ascontiguousarray(np.asarray(q, dtype=np.float32))
    k = np.ascontiguousarray(np.asarray(k, dtype=np.float32))
    v = np.ascontiguousarray(np.asarray(v, dtype=np.float32))
    B, H, S_, D_ = q.shape
    assert (S_, D_) == (S, D), (S_, D_)
    G = B * H
    per = G // NCORES
    assert per == NH

    qf = q.reshape(G, S, D)
    kf = k.reshape(G, S, D)
    vf = v.reshape(G, S, D)
    in_maps = [
        {
            "q": np.ascontiguousarray(qf[i * per:(i + 1) * per]),
            "k": np.ascontiguousarray(kf[i * per:(i + 1) * per]),
            "v": np.ascontiguousarray(vf[i * per:(i + 1) * per]),
        }
        for i in range(NCORES)
    ]

    results = _run(in_maps, trace=_trace)

    out = np.empty((G, S, D), np.float32)
    attn = np.zeros((G, S, S), np.float32)
    for i in range(NCORES):
        out[i * per:(i + 1) * per] = results[i]["out"]
        bandr = results[i]["band"]  # [NH, T, P, W]
        for g in range(per):
            gi = i * per + g
            for t in range(T):
                w0 = max(0, t - 2) * P
                wN = (t + 1) * P - w0 if t < 2 else W
                attn[gi, t * P:(t + 1) * P, w0:w0 + wN] = bandr[g, t, :, :wN]
    return out.reshape(B, H, S, D), attn.reshape(B, H, S, S)
